# revision 1
# baseline (speedup 1.0000x reference)
"""Multi-head attention (RoPE, causal) Trainium2 Bass kernel.

Problem: x[2,2048,1024], Wqkv[3072,1024], Wproj[1024,1024], H=16 heads, D=64.
Sharding: 8 cores = (2 batches) x (4 head-groups of 4 heads).  Each core
computes qkv + rope + causal attention + its partial output projection for its
4 heads; the host sums the 4 partial projections per batch.

Layout strategy (all matmuls in float32r, full PE rate at N>=256):
  - Host passes x transposed (xT [C,T]) so Q,K are produced directly in
    [d, t] layout (lhsT=W.T, rhs=xT) and V in natural [t, d] layout
    (lhsT=xT, rhs=Wv.T).  No on-chip transposes anywhere.
  - Attention runs in "S-transposed" orientation: S_T[k_pos, q] = K.T @ Q,
    2 heads row-packed per matmul pair (K=64 contraction each).
  - Softmax: no max-subtraction (logits are O(5)); exp on ScalarE with
    scale=1/8 folded in; causal band masked by adding -3e4 on the diagonal
    128x128 blocks; fully-masked columns excluded via matmul subranges.
  - P@V accumulates out_T[d, q] with lhsT=V (natural layout); denominator
    via a ones-column in V with M=1 matmuls into separate PSUM banks.
  - Projection: lhsT=ctx_T (stationary), rhs=Wproj slice -> natural [t, o]
    partial output, DMA'd out; host reduces.
"""

import numpy as np

import concourse.bass as bass
import concourse.mybir as mybir
import concourse.tile as tile

F32 = mybir.dt.float32
F32R = mybir.dt.float32r
BF16 = mybir.dt.bfloat16

# per-stage matmul dtypes: attention tolerates bf16 (64-wide dots, errors
# ~2^-9 relative); bf16 matmuls stream ~2.65x faster than fp32r on TRN2.
ATT_BF16 = False
QKV_BF16 = False
PROJ_BF16 = False
ATT_DT = BF16 if ATT_BF16 else F32R
QKV_DT = BF16 if QKV_BF16 else F32R
PROJ_DT = BF16 if PROJ_BF16 else F32R
EXP = mybir.ActivationFunctionType.Exp

B, T, C, H, D = 2, 2048, 1024, 16, 64
HL = H // 4          # 4 heads per core
N_CORES = 8
ROPE_BASE = 10000.0
SCALE = float(D) ** -0.5
NEG = -30000.0
TT = 512             # t-tile / q-tile size
NTT = T // TT        # 4
KB = 128             # k block
NKB = T // KB        # 16


# ---------------------------------------------------------------- legalizer
_wfx = [0]


def _legalize_sync_waits(nc, limit=1):
    """walrus in this container accepts only `limit` sync-waits per
    instruction; move excess waits onto preceding same-engine NOPs."""
    n_fixed = 0
    for f in nc.m.functions:
        for blk in f.blocks:
            insts = blk.instructions
            new_list = []
            changed = False
            for inst in insts:
                si = inst.sync_info
                if si is not None and len(si.on_wait) > limit:
                    waits = list(si.on_wait)
                    keep = waits[-limit:]
                    excess = waits[:-limit]
                    for k in range(0, len(excess), limit):
                        _wfx[0] += 1
                        new_list.append(mybir.InstNoOp(
                            name=f"waitfix_{_wfx[0]}",
                            engine=inst.engine,
                            bass_nofuse=True,
                            sync_info=mybir.SyncInfo(
                                on_wait=excess[k:k + limit], on_update=[]),
                        ))
                    si.on_wait = keep
                    changed = True
                    n_fixed += 1
                new_list.append(inst)
            if changed:
                insts.clear()
                insts.extend(new_list)
    return n_fixed


# ---------------------------------------------------------------- bass build
DEFAULT_CFG = dict(shift="sync", qc="vector", qs="gpsimd", add="vector",
                   norm="gpsimd", ctxB_evac="scalar", denA_evac="scalar",
                   denB_evac="vector", raw="vector")


def build_bass(loop_n=1, cfg=None):
    cfg = dict(DEFAULT_CFG, **(cfg or {}))

    def eng(name):
        return getattr(nc, cfg[name])

    nc = bass.Bass("TRN2")
    xT = nc.dram_tensor("xT", [C, T], QKV_DT, kind="ExternalInput")
    wqkT = nc.dram_tensor("wqkT", [C, 2 * HL * D], QKV_DT, kind="ExternalInput")
    wvT = nc.dram_tensor("wvT", [C, HL * D], QKV_DT, kind="ExternalInput")
    wpT = nc.dram_tensor("wpT", [HL * D, C], PROJ_DT, kind="ExternalInput")
    cosF = nc.dram_tensor("cosF", [128, T], F32, kind="ExternalInput")
    sinF = nc.dram_tensor("sinF", [128, T], F32, kind="ExternalInput")
    maskband = nc.dram_tensor("maskband", [128, 128], F32, kind="ExternalInput")
    vones = nc.dram_tensor("vones", [1, NKB * HL], ATT_DT, kind="ExternalInput")
    y = nc.dram_tensor("y", [T, C], F32, kind="ExternalOutput")

    import contextlib

    def _cp(engine_name):
        if engine_name == "scalar":
            return nc.scalar.copy
        return getattr(nc, engine_name).tensor_copy

    with tile.TileContext(nc) as tc:
        with tc.tile_pool(name="persist", bufs=1) as persist:
            # -- persistent tiles --
            # wqk split lo/hi so the first QKV matmuls start after 2MB lands
            wqk_lo = persist.tile([128, 4, 512], QKV_DT, tag="wqk_lo")
            wqk_hi = persist.tile([128, 4, 512], QKV_DT, tag="wqk_hi")
            wv_sb = persist.tile([128, 8, 256], QKV_DT, tag="wv")
            cos_sb = persist.tile([128, T], F32, tag="cos")
            sin_sb = persist.tile([128, T], F32, tag="sin")
            mask_sb = persist.tile([128, 128], F32, tag="mask")
            wp_sb = persist.tile([128, 2, 1024], PROJ_DT, tag="wp")
            qtp = [[persist.tile([128, TT], ATT_DT, tag=f"qt{i}_{t}",
                                 name=f"qt{i}_{t}") for t in range(NTT)]
                   for i in range(2)]
            ktp = [[persist.tile([128, TT], ATT_DT, tag=f"kt{i}_{t}",
                                 name=f"kt{i}_{t}") for t in range(NTT)]
                   for i in range(2)]
            ctx_sb = [persist.tile([128, T], PROJ_DT, tag=f"ctx{i}",
                                   name=f"ctx{i}") for i in range(2)]
            # V natural + trailing ones column: [t, tb, head, 65]
            vp = [persist.tile([128, 4, HL, D + 1], ATT_DT, tag=f"v{t}",
                               name=f"v{t}") for t in range(NTT)]

            wqkT_r = wqkT.rearrange("(co p) o -> p co o", p=128)
            nc.sync.dma_start(out=wqk_lo, in_=wqkT_r[:, 0:4, :])

            loop_cm = (tc.For_i(0, loop_n, 1) if loop_n > 1
                       else contextlib.nullcontext())
            with loop_cm:
                # psum banks: qkv 2, sAB 2x2, ctxA 1, ctxB 1 -> 8 total
                with tc.tile_pool(name="xpool", bufs=2) as xpool, \
                     tc.tile_pool(name="ropetmp", bufs=2) as rpool, \
                     tc.tile_pool(name="ptpool", bufs=3) as ptpool, \
                     tc.tile_pool(name="dramp", bufs=2, space="DRAM") as dramp, \
                     tc.tile_pool(name="npool", bufs=1) as npool, \
                     tc.tile_pool(name="bcps", bufs=1, space="PSUM") as bcps:

                    den = {}

                    xtiles = {}

                    def emit_x(tt):
                        ts = slice(tt * TT, (tt + 1) * TT)
                        x_lo = xpool.tile([128, 4, TT], QKV_DT, tag="x_lo",
                                          name="x_lo")
                        x_hi = xpool.tile([128, 4, TT], QKV_DT, tag="x_hi",
                                          name="x_hi")
                        xT_r = xT.rearrange("(co p) t -> p co t", p=128)
                        nc.sync.dma_start(out=x_lo, in_=xT_r[:, 0:4, ts])
                        if tt == 0:
                            nc.sync.dma_start(out=wqk_hi,
                                              in_=wqkT_r[:, 4:8, :])
                        nc.sync.dma_start(out=x_hi, in_=xT_r[:, 4:8, ts])
                        if tt == 0:
                            nc.sync.dma_start(out=cos_sb, in_=cosF[:, :])
                            nc.sync.dma_start(out=sin_sb, in_=sinF[:, :])
                        xtiles[tt] = (x_lo, x_hi)

                    def emit_qkv_ttile(tt):
                        ts = slice(tt * TT, (tt + 1) * TT)
                        x_lo, x_hi = xtiles.pop(tt)
                        if tt == 0:
                            # deferred loads, ordered by first-use time
                            nc.sync.dma_start(
                                out=wv_sb,
                                in_=wvT.rearrange("(co p) o -> p co o", p=128))
                            nc.sync.dma_start(out=mask_sb, in_=maskband[:, :])
                            for t_ in range(NTT):
                                nc.sync.dma_start(
                                    out=vp[t_][:, :, :, D:D + 1],
                                    in_=vones[0:1, 0:4 * HL]
                                    .partition_broadcast(128))
                        if tt == 2:
                            # wp is only needed by the projection at the end
                            nc.sync.dma_start(
                                out=wp_sb,
                                in_=wpT.rearrange("(kb p) o -> p kb o", p=128))

                        def xc(c):
                            return (x_lo if c < 4 else x_hi)[:, c % 4, :]

                        def wc(c, osl):
                            return (wqk_lo if c < 4 else wqk_hi)[:, c % 4, osl]

                        for ob in (0, 2, 1, 3):
                            qk_ps = bcps.tile([128, TT], F32, tag="qkv",
                                              bufs=2, name="qk_ps")
                            for c in range(8):
                                nc.tensor.matmul(
                                    qk_ps[:, :],
                                    wc(c, slice(ob * 128, (ob + 1) * 128)),
                                    xc(c), start=(c == 0), stop=(c == 7))
                            dst = (qtp if ob < 2 else ktp)[ob % 2][tt]
                            # rope: dst = raw*cos + shift32(raw)*sin_signed
                            raw = rpool.tile([128, TT], F32, tag="raw")
                            eng("raw").tensor_copy(raw[:, :], qk_ps[:, :])
                            qc = rpool.tile([128, TT], F32, tag="qc")
                            eng("qc").tensor_mul(qc[:, :], raw[:, :],
                                                 cos_sb[:, ts])
                            # partition shift p -> p^32 as 2 structured DMAs
                            tmp = rpool.tile([128, TT], F32, tag="tmp")
                            for h2 in range(2):
                                b0 = h2 * 64
                                eng("shift").dma_start(
                                    out=tmp[b0:b0 + 32, :],
                                    in_=raw[b0 + 32:b0 + 64, :])
                                eng("shift").dma_start(
                                    out=tmp[b0 + 32:b0 + 64, :],
                                    in_=raw[b0:b0 + 32, :])
                            qs = rpool.tile([128, TT], F32, tag="qs")
                            eng("qs").tensor_mul(qs[:, :], tmp[:, :],
                                                 sin_sb[:, ts])
                            eng("add").tensor_add(dst[:, :], qc[:, :],
                                                  qs[:, :])
                        for tb in range(4):
                            v_ps = bcps.tile([128, HL * D], F32, tag="qkv",
                                             bufs=2, name="v_ps")
                            for c in range(8):
                                nc.tensor.matmul(
                                    v_ps[:, :],
                                    (x_lo if c < 4 else x_hi)[
                                        :, c % 4, tb * 128:(tb + 1) * 128],
                                    wv_sb[:, c, :],
                                    start=(c == 0), stop=(c == 7))
                            nc.scalar.copy(
                                vp[tt][:, tb, :, 0:D],
                                v_ps[:, :].rearrange("p (h d) -> p h d", d=D))

                    def emit_attention_qtile(p, qi):
                        hA, hB = 2 * p, 2 * p + 1
                        qsl = slice(qi * TT, (qi + 1) * TT)
                        nkb = 4 * (qi + 1)
                        qtile = qtp[p][qi]
                        if p == 0:
                            # per-qtile denominator staging: head (p, A/B) in
                            # row 64*p + 32*half; memset 1.0 keeps unused
                            # rows' reciprocals finite
                            den[qi] = npool.tile([128, TT], F32, tag="denq",
                                                 bufs=2, name="denq")
                            nc.vector.memset(den[qi][:, :], 1.0)
                        ctxA = bcps.tile([65, TT], F32, tag="ctxA",
                                         name="ctxA")
                        ctxB = bcps.tile([65, TT], F32, tag="ctxB",
                                         name="ctxB")
                        for j0 in range(0, nkb, 2):
                            tiles = []
                            for j in (j0, j0 + 1):
                                kpiece = ktp[p][j // 4]
                                ksl = slice((j % 4) * KB, (j % 4 + 1) * KB)
                                sAB = bcps.tile([128, 2 * TT], F32,
                                                tag="sAB", bufs=2, name="sAB")
                                ptAB = ptpool.tile([128, 2 * TT], ATT_DT,
                                                   tag="ptAB", name="ptAB")
                                nc.tensor.matmul(
                                    sAB[:, 0:TT], kpiece[0:64, ksl],
                                    qtile[0:64, :], start=True, stop=True,
                                    tile_position=(0, 0))
                                nc.tensor.matmul(
                                    sAB[:, TT:2 * TT], kpiece[64:128, ksl],
                                    qtile[64:128, :], start=True, stop=True,
                                    tile_position=(64, 0))
                                delta = j * KB - qi * TT
                                d0 = max(0, delta)
                                if 0 <= delta < TT:
                                    for half in (0, TT):
                                        bs = slice(half + delta,
                                                   half + delta + KB)
                                        nc.vector.tensor_add(
                                            sAB[:, bs], sAB[:, bs],
                                            mask_sb[:, :])
                                    for half in (0, TT):
                                        es = slice(half + d0, half + TT)
                                        nc.scalar.activation(
                                            ptAB[:, es], sAB[:, es], EXP,
                                            scale=SCALE)
                                else:
                                    nc.scalar.activation(
                                        ptAB[:, :], sAB[:, :], EXP,
                                        scale=SCALE)
                                tiles.append((j, d0, ptAB))
                            for j, d0, ptAB in tiles:
                                st, sp = (j == 0), (j == nkb - 1)
                                sub = slice(d0, TT)
                                vpc = vp[j // 4]
                                nc.tensor.matmul(
                                    ctxA[0:65, sub],
                                    vpc[:, j % 4, hA, 0:D + 1],
                                    ptAB[:, d0:TT], start=st, stop=sp)
                                nc.tensor.matmul(
                                    ctxB[0:65, sub],
                                    vpc[:, j % 4, hB, 0:D + 1],
                                    ptAB[:, TT + d0:2 * TT],
                                    start=st, stop=sp)
                        # evacuate unnormalized ctx + denominators
                        # (partition-shifted engine copies)
                        nc.vector.tensor_copy(ctx_sb[p][0:64, qsl],
                                              ctxA[0:64, :])
                        _cp(cfg["ctxB_evac"])(ctx_sb[p][64:128, qsl],
                                               ctxB[0:64, :])
                        _cp(cfg["denA_evac"])(den[qi][64 * p:64 * p + 1, :],
                                              ctxA[64:65, :])
                        _cp(cfg["denB_evac"])(den[qi]
                                              [64 * p + 32:64 * p + 33, :],
                                              ctxB[64:65, :])

                    def emit_normalize(qi):
                        qsl = slice(qi * TT, (qi + 1) * TT)
                        rec = npool.tile([128, TT], F32, tag="rec", bufs=2,
                                         name="rec")
                        nc.vector.reciprocal(rec[:, :], den[qi][:, :])
                        # bounce through DRAM to partition-broadcast
                        recd = dramp.tile([4, TT], F32, tag="recd",
                                          name="recd")
                        rec_r = rec.rearrange("(a b) f -> a b f", b=32)
                        nc.sync.dma_start(out=recd[0:4, :],
                                          in_=rec_r[0:4, 0, :])
                        for p in range(2):
                            rb = npool.tile([128, TT], F32, tag=f"rb{p}",
                                            bufs=2, name=f"rb{p}")
                            nc.sync.dma_start(
                                out=rb[0:64, :],
                                in_=recd[2 * p:2 * p + 1, :]
                                .partition_broadcast(64))
                            nc.sync.dma_start(
                                out=rb[64:128, :],
                                in_=recd[2 * p + 1:2 * p + 2, :]
                                .partition_broadcast(64))
                            eng("norm").tensor_mul(ctx_sb[p][:, qsl],
                                                   ctx_sb[p][:, qsl],
                                                   rb[:, :])

                    # interleave: QKV ttile tt, then attention qtile tt for
                    # both pairs (causality: qtile tt needs pieces <= tt only)
                    emit_x(0)
                    for tt in range(NTT):
                        if tt + 1 < NTT:
                            emit_x(tt + 1)
                        emit_qkv_ttile(tt)
                        emit_attention_qtile(0, tt)
                        emit_attention_qtile(1, tt)
                        emit_normalize(tt)

                # ---------------- Phase D: output projection ----------------
                with tc.tile_pool(name="yps", bufs=2, space="PSUM") as yps, \
                     tc.tile_pool(name="ysb", bufs=3) as ysb:
                    for tb in range(NKB):
                        tsl = slice(tb * 128, (tb + 1) * 128)
                        y_ps = yps.tile([128, 1024], F32, tag="y_ps")
                        for no in range(2):
                            osl = slice(no * 512, (no + 1) * 512)
                            for kb in range(2):
                                nc.tensor.matmul(
                                    y_ps[:, osl], ctx_sb[kb][:, tsl],
                                    wp_sb[:, kb, osl],
                                    start=(kb == 0), stop=(kb == 1))
                        y_sb = ysb.tile([128, 1024], F32, tag="y_sb")
                        nc.vector.tensor_copy(y_sb[:, 0:512], y_ps[:, 0:512])
                        nc.scalar.copy(y_sb[:, 512:1024], y_ps[:, 512:1024])
                        nc.sync.dma_start(out=y[tsl, :], in_=y_sb[:, :])

    return nc


# ---------------------------------------------------------------- host side
def _rope_tables():
    inv = 1.0 / (ROPE_BASE ** (np.arange(0, D, 2, dtype=np.float32) / D))  # [32]
    t = np.arange(T, dtype=np.float32)
    freqs = np.outer(t, inv)                      # [T, 32]
    cosF = np.empty((128, T), dtype=np.float32)
    sinF = np.empty((128, T), dtype=np.float32)
    for p_ in range(128):
        d = p_ % D
        cosF[p_] = np.cos(freqs[:, d % 32])
        s = np.sin(freqs[:, d % 32])
        sinF[p_] = -s if d < 32 else s
    return cosF, sinF


def _mask_band():
    jp = np.arange(128)[:, None]
    qb = np.arange(128)[None, :]
    return np.where(qb >= jp, 0.0, NEG).astype(np.float32)


def _np_dt(dt):
    import ml_dtypes
    return ml_dtypes.bfloat16 if dt == BF16 else np.float32


def prepare_in_maps(x, Wqkv, Wproj):
    x = np.asarray(x, dtype=np.float32)
    Wqkv = np.asarray(Wqkv, dtype=np.float32)
    Wproj = np.asarray(Wproj, dtype=np.float32)
    qkv_np, proj_np, att_np = _np_dt(QKV_DT), _np_dt(PROJ_DT), _np_dt(ATT_DT)
    cosF, sinF = _rope_tables()
    mb = _mask_band()
    xTs = [np.ascontiguousarray(x[b].T).astype(qkv_np) for b in range(B)]
    in_maps = []
    for core in range(N_CORES):
        b, g = divmod(core, 4)
        hs = [4 * g + i for i in range(HL)]
        q_rows = np.concatenate([Wqkv[h * D:(h + 1) * D] for h in hs])
        k_rows = np.concatenate([Wqkv[C + h * D:C + (h + 1) * D] for h in hs])
        v_rows = np.concatenate([Wqkv[2 * C + h * D:2 * C + (h + 1) * D] for h in hs])
        wqkT = np.ascontiguousarray(np.concatenate([q_rows, k_rows]).T).astype(qkv_np)
        wvT = np.ascontiguousarray(v_rows.T).astype(qkv_np)
        cols = np.concatenate([np.arange(h * D, (h + 1) * D) for h in hs])
        wpT = np.ascontiguousarray(Wproj[:, cols].T).astype(proj_np)
        in_maps.append(dict(xT=xTs[b], wqkT=wqkT, wvT=wvT, wpT=wpT,
                            cosF=cosF, sinF=sinF, maskband=mb,
                            vones=np.ones((1, NKB * HL), dtype=att_np)))
    return in_maps


_CACHE = {}


def _get_nc():
    if "nc" not in _CACHE:
        _CACHE["nc"] = build_bass()
    return _CACHE["nc"]


def _make_runner(nc, chain=1):
    """Jitted SPMD executor for an arbitrary prebuilt Bass module."""
    import jax
    from jax.experimental.shard_map import shard_map
    from jax.sharding import Mesh, PartitionSpec
    from concourse import bass2jax

    bass2jax.install_neuronx_cc_hook()

    import concourse.mybir as _mb
    partition_name = (nc.partition_id_tensor.name
                      if nc.partition_id_tensor else None)
    in_names, out_names, out_avals, zero_outs = [], [], [], []
    for alloc in nc.m.functions[0].allocations:
        if not isinstance(alloc, _mb.MemoryLocationSet):
            continue
        name = alloc.memorylocations[0].name
        if alloc.kind == "ExternalInput":
            if name != partition_name:
                in_names.append(name)
        elif alloc.kind == "ExternalOutput":
            out_names.append(name)
            shape = tuple(alloc.tensor_shape)
            dtype = _mb.dt.np(alloc.dtype)
            out_avals.append(jax.core.ShapedArray(shape, dtype))
            zero_outs.append(np.zeros(shape, dtype))
    n_params = len(in_names)
    all_in_names = in_names + out_names
    if partition_name is not None:
        all_in_names = all_in_names + [partition_name]

    def _body(*args):
        params = list(args[:n_params])
        bufs = list(args[n_params:])
        for _ in range(chain):
            operands = params + bufs
            if partition_name is not None:
                operands.append(bass2jax.partition_id_tensor())
            outs = bass2jax._bass_exec_p.bind(
                *operands,
                out_avals=tuple(out_avals),
                in_names=tuple(all_in_names),
                out_names=tuple(out_names),
                lowering_input_output_aliases=(),
                sim_require_finite=True,
                sim_require_nnan=True,
                nc=nc,
            )
            bufs = list(outs)
        return tuple(bufs)

    devices = jax.devices()[:N_CORES]
    mesh = Mesh(np.array(devices), ("core",))
    n_outs = len(out_names)
    in_specs = (PartitionSpec("core"),) * (n_params + n_outs)
    out_specs = (PartitionSpec("core"),) * n_outs
    fn = jax.jit(shard_map(_body, mesh=mesh, in_specs=in_specs,
                           out_specs=out_specs, check_rep=False),
                 keep_unused=True)
    return dict(fn=fn, in_names=in_names, out_names=out_names,
                out_avals=out_avals, zero_outs=zero_outs, n_params=n_params)


def _get_runner(chain=1):
    key = ("runner", chain)
    if key in _CACHE:
        return _CACHE[key]
    nc = _get_nc()
    if not _CACHE.get("legalized"):
        _legalize_sync_waits(nc, limit=1)
        _CACHE["legalized"] = True
    runner = _make_runner(nc, chain)
    _CACHE[key] = runner
    return runner


def _concat_args(runner, in_maps):
    concat_in = [np.concatenate([np.asarray(m[name]) for m in in_maps], axis=0)
                 for name in runner["in_names"]]
    concat_zeros = [np.zeros((N_CORES * z.shape[0], *z.shape[1:]), z.dtype)
                    for z in runner["zero_outs"]]
    return concat_in + concat_zeros


def _run(in_maps, chain=1, args=None):
    runner = _get_runner(chain)
    if args is None:
        args = _concat_args(runner, in_maps)
    out_arrs = runner["fn"](*args)
    results = []
    for c in range(N_CORES):
        results.append({
            name: np.asarray(out_arrs[i]).reshape(
                N_CORES, *runner["out_avals"][i].shape)[c]
            for i, name in enumerate(runner["out_names"])})
    return results


def kernel(x, Wqkv, Wproj):
    in_maps = prepare_in_maps(x, Wqkv, Wproj)
    results = _run(in_maps)
    out = np.zeros((B, T, C), dtype=np.float32)
    for core in range(N_CORES):
        b = core // 4
        out[b] += results[core]["y"]
    return out



# revision 2
# speedup vs baseline: 1.3483x; 1.3483x over previous
"""Multi-head attention (RoPE, causal) Trainium2 Bass kernel.

Problem: x[2,2048,1024], Wqkv[3072,1024], Wproj[1024,1024], H=16 heads, D=64.
Sharding: 8 cores = (2 batches) x (4 head-groups of 4 heads).  Each core
computes qkv + rope + causal attention + its partial output projection for its
4 heads; the host sums the 4 partial projections per batch.

Layout strategy (all matmuls in float32r, full PE rate at N>=256):
  - Host passes x transposed (xT [C,T]) so Q,K are produced directly in
    [d, t] layout (lhsT=W.T, rhs=xT) and V in natural [t, d] layout
    (lhsT=xT, rhs=Wv.T).  No on-chip transposes anywhere.
  - Attention runs in "S-transposed" orientation: S_T[k_pos, q] = K.T @ Q,
    2 heads row-packed per matmul pair (K=64 contraction each).
  - Softmax: no max-subtraction (logits are O(5)); exp on ScalarE with
    scale=1/8 folded in; causal band masked by adding -3e4 on the diagonal
    128x128 blocks; fully-masked columns excluded via matmul subranges.
  - P@V accumulates out_T[d, q] with lhsT=V (natural layout); denominator
    via a ones-column in V with M=1 matmuls into separate PSUM banks.
  - Projection: lhsT=ctx_T (stationary), rhs=Wproj slice -> natural [t, o]
    partial output, DMA'd out; host reduces.
"""

import numpy as np

import concourse.bass as bass
import concourse.mybir as mybir
import concourse.tile as tile

F32 = mybir.dt.float32
F32R = mybir.dt.float32r
BF16 = mybir.dt.bfloat16

# per-stage matmul dtypes: attention tolerates bf16 (64-wide dots, errors
# ~2^-9 relative); bf16 matmuls stream ~2.65x faster than fp32r on TRN2.
ATT_BF16 = True
QKV_BF16 = True
PROJ_BF16 = True
ATT_DT = BF16 if ATT_BF16 else F32R
QKV_DT = BF16 if QKV_BF16 else F32R
PROJ_DT = BF16 if PROJ_BF16 else F32R
EXP = mybir.ActivationFunctionType.Exp

B, T, C, H, D = 2, 2048, 1024, 16, 64
HL = H // 4          # 4 heads per core
N_CORES = 8
ROPE_BASE = 10000.0
SCALE = float(D) ** -0.5
NEG = -30000.0
TT = 512             # t-tile / q-tile size
NTT = T // TT        # 4
KB = 128             # k block
NKB = T // KB        # 16


# ---------------------------------------------------------------- legalizer
_wfx = [0]


def _legalize_sync_waits(nc, limit=1):
    """walrus in this container accepts only `limit` sync-waits per
    instruction; move excess waits onto preceding same-engine NOPs."""
    n_fixed = 0
    for f in nc.m.functions:
        for blk in f.blocks:
            insts = blk.instructions
            new_list = []
            changed = False
            for inst in insts:
                si = inst.sync_info
                if si is not None and len(si.on_wait) > limit:
                    waits = list(si.on_wait)
                    keep = waits[-limit:]
                    excess = waits[:-limit]
                    for k in range(0, len(excess), limit):
                        _wfx[0] += 1
                        new_list.append(mybir.InstNoOp(
                            name=f"waitfix_{_wfx[0]}",
                            engine=inst.engine,
                            bass_nofuse=True,
                            sync_info=mybir.SyncInfo(
                                on_wait=excess[k:k + limit], on_update=[]),
                        ))
                    si.on_wait = keep
                    changed = True
                    n_fixed += 1
                new_list.append(inst)
            if changed:
                insts.clear()
                insts.extend(new_list)
    return n_fixed


# ---------------------------------------------------------------- bass build
DEFAULT_CFG = dict(shift="sync", qc="vector", qs="gpsimd", add="vector",
                   norm="gpsimd", ctxB_evac="scalar", denA_evac="scalar",
                   denB_evac="vector", raw="vector")


def build_bass(loop_n=1, cfg=None):
    cfg = dict(DEFAULT_CFG, **(cfg or {}))

    def eng(name):
        return getattr(nc, cfg[name])

    nc = bass.Bass("TRN2")
    xT = nc.dram_tensor("xT", [C, T], QKV_DT, kind="ExternalInput")
    wqkT = nc.dram_tensor("wqkT", [C, 2 * HL * D], QKV_DT, kind="ExternalInput")
    wvT = nc.dram_tensor("wvT", [C, HL * D], QKV_DT, kind="ExternalInput")
    wpT = nc.dram_tensor("wpT", [HL * D, C], PROJ_DT, kind="ExternalInput")
    cosF = nc.dram_tensor("cosF", [128, T], F32, kind="ExternalInput")
    sinF = nc.dram_tensor("sinF", [128, T], F32, kind="ExternalInput")
    maskband = nc.dram_tensor("maskband", [128, 128], F32, kind="ExternalInput")
    vones = nc.dram_tensor("vones", [1, NKB * HL], ATT_DT, kind="ExternalInput")
    y = nc.dram_tensor("y", [T, C], F32, kind="ExternalOutput")

    import contextlib

    def _cp(engine_name):
        if engine_name == "scalar":
            return nc.scalar.copy
        return getattr(nc, engine_name).tensor_copy

    with tile.TileContext(nc) as tc:
        with tc.tile_pool(name="persist", bufs=1) as persist:
            # -- persistent tiles --
            # wqk split lo/hi so the first QKV matmuls start after 2MB lands
            wqk_lo = persist.tile([128, 4, 512], QKV_DT, tag="wqk_lo")
            wqk_hi = persist.tile([128, 4, 512], QKV_DT, tag="wqk_hi")
            wv_sb = persist.tile([128, 8, 256], QKV_DT, tag="wv")
            cos_sb = persist.tile([128, T], F32, tag="cos")
            sin_sb = persist.tile([128, T], F32, tag="sin")
            mask_sb = persist.tile([128, 128], F32, tag="mask")
            wp_sb = persist.tile([128, 2, 1024], PROJ_DT, tag="wp")
            qtp = [[persist.tile([128, TT], ATT_DT, tag=f"qt{i}_{t}",
                                 name=f"qt{i}_{t}") for t in range(NTT)]
                   for i in range(2)]
            ktp = [[persist.tile([128, TT], ATT_DT, tag=f"kt{i}_{t}",
                                 name=f"kt{i}_{t}") for t in range(NTT)]
                   for i in range(2)]
            ctx_sb = [persist.tile([128, T], PROJ_DT, tag=f"ctx{i}",
                                   name=f"ctx{i}") for i in range(2)]
            # V natural + trailing ones column: [t, tb, head, 65]
            vp = [persist.tile([128, 4, HL, D + 1], ATT_DT, tag=f"v{t}",
                               name=f"v{t}") for t in range(NTT)]

            wqkT_r = wqkT.rearrange("(co p) o -> p co o", p=128)
            nc.sync.dma_start(out=wqk_lo, in_=wqkT_r[:, 0:4, :])

            loop_cm = (tc.For_i(0, loop_n, 1) if loop_n > 1
                       else contextlib.nullcontext())
            with loop_cm:
                # psum banks: qkv 2, sAB 2x2, ctxA 1, ctxB 1 -> 8 total
                with tc.tile_pool(name="xpool", bufs=2) as xpool, \
                     tc.tile_pool(name="ropetmp", bufs=2) as rpool, \
                     tc.tile_pool(name="ptpool", bufs=3) as ptpool, \
                     tc.tile_pool(name="dramp", bufs=2, space="DRAM") as dramp, \
                     tc.tile_pool(name="npool", bufs=1) as npool, \
                     tc.tile_pool(name="bcps", bufs=1, space="PSUM") as bcps:

                    den = {}

                    xtiles = {}

                    def emit_x(tt):
                        ts = slice(tt * TT, (tt + 1) * TT)
                        x_lo = xpool.tile([128, 4, TT], QKV_DT, tag="x_lo",
                                          name="x_lo")
                        x_hi = xpool.tile([128, 4, TT], QKV_DT, tag="x_hi",
                                          name="x_hi")
                        xT_r = xT.rearrange("(co p) t -> p co t", p=128)
                        nc.sync.dma_start(out=x_lo, in_=xT_r[:, 0:4, ts])
                        if tt == 0:
                            nc.sync.dma_start(out=wqk_hi,
                                              in_=wqkT_r[:, 4:8, :])
                        nc.sync.dma_start(out=x_hi, in_=xT_r[:, 4:8, ts])
                        if tt == 0:
                            nc.sync.dma_start(out=cos_sb, in_=cosF[:, :])
                            nc.sync.dma_start(out=sin_sb, in_=sinF[:, :])
                        xtiles[tt] = (x_lo, x_hi)

                    def emit_qkv_ttile(tt):
                        ts = slice(tt * TT, (tt + 1) * TT)
                        x_lo, x_hi = xtiles.pop(tt)
                        if tt == 0:
                            # deferred loads, ordered by first-use time
                            nc.sync.dma_start(
                                out=wv_sb,
                                in_=wvT.rearrange("(co p) o -> p co o", p=128))
                            nc.sync.dma_start(out=mask_sb, in_=maskband[:, :])
                            for t_ in range(NTT):
                                nc.sync.dma_start(
                                    out=vp[t_][:, :, :, D:D + 1],
                                    in_=vones[0:1, 0:4 * HL]
                                    .partition_broadcast(128))
                        if tt == 2:
                            # wp is only needed by the projection at the end
                            nc.sync.dma_start(
                                out=wp_sb,
                                in_=wpT.rearrange("(kb p) o -> p kb o", p=128))

                        def xc(c):
                            return (x_lo if c < 4 else x_hi)[:, c % 4, :]

                        def wc(c, osl):
                            return (wqk_lo if c < 4 else wqk_hi)[:, c % 4, osl]

                        for ob in (0, 2, 1, 3):
                            qk_ps = bcps.tile([128, TT], F32, tag="qkv",
                                              bufs=2, name="qk_ps")
                            for c in range(8):
                                nc.tensor.matmul(
                                    qk_ps[:, :],
                                    wc(c, slice(ob * 128, (ob + 1) * 128)),
                                    xc(c), start=(c == 0), stop=(c == 7))
                            dst = (qtp if ob < 2 else ktp)[ob % 2][tt]
                            # rope: dst = raw*cos + shift32(raw)*sin_signed
                            raw = rpool.tile([128, TT], F32, tag="raw")
                            eng("raw").tensor_copy(raw[:, :], qk_ps[:, :])
                            qc = rpool.tile([128, TT], F32, tag="qc")
                            eng("qc").tensor_mul(qc[:, :], raw[:, :],
                                                 cos_sb[:, ts])
                            # partition shift p -> p^32 as 2 structured DMAs
                            tmp = rpool.tile([128, TT], F32, tag="tmp")
                            for h2 in range(2):
                                b0 = h2 * 64
                                eng("shift").dma_start(
                                    out=tmp[b0:b0 + 32, :],
                                    in_=raw[b0 + 32:b0 + 64, :])
                                eng("shift").dma_start(
                                    out=tmp[b0 + 32:b0 + 64, :],
                                    in_=raw[b0:b0 + 32, :])
                            qs = rpool.tile([128, TT], F32, tag="qs")
                            eng("qs").tensor_mul(qs[:, :], tmp[:, :],
                                                 sin_sb[:, ts])
                            eng("add").tensor_add(dst[:, :], qc[:, :],
                                                  qs[:, :])
                        for tb in range(4):
                            v_ps = bcps.tile([128, HL * D], F32, tag="qkv",
                                             bufs=2, name="v_ps")
                            for c in range(8):
                                nc.tensor.matmul(
                                    v_ps[:, :],
                                    (x_lo if c < 4 else x_hi)[
                                        :, c % 4, tb * 128:(tb + 1) * 128],
                                    wv_sb[:, c, :],
                                    start=(c == 0), stop=(c == 7))
                            nc.scalar.copy(
                                vp[tt][:, tb, :, 0:D],
                                v_ps[:, :].rearrange("p (h d) -> p h d", d=D))

                    def emit_attention_qtile(p, qi):
                        hA, hB = 2 * p, 2 * p + 1
                        qsl = slice(qi * TT, (qi + 1) * TT)
                        nkb = 4 * (qi + 1)
                        qtile = qtp[p][qi]
                        if p == 0:
                            # per-qtile denominator staging: head (p, A/B) in
                            # row 64*p + 32*half; memset 1.0 keeps unused
                            # rows' reciprocals finite
                            den[qi] = npool.tile([128, TT], F32, tag="denq",
                                                 bufs=2, name="denq")
                            nc.vector.memset(den[qi][:, :], 1.0)
                        ctxA = bcps.tile([65, TT], F32, tag="ctxA",
                                         name="ctxA")
                        ctxB = bcps.tile([65, TT], F32, tag="ctxB",
                                         name="ctxB")
                        for j0 in range(0, nkb, 2):
                            tiles = []
                            for j in (j0, j0 + 1):
                                kpiece = ktp[p][j // 4]
                                ksl = slice((j % 4) * KB, (j % 4 + 1) * KB)
                                sAB = bcps.tile([128, 2 * TT], F32,
                                                tag="sAB", bufs=2, name="sAB")
                                ptAB = ptpool.tile([128, 2 * TT], ATT_DT,
                                                   tag="ptAB", name="ptAB")
                                nc.tensor.matmul(
                                    sAB[:, 0:TT], kpiece[0:64, ksl],
                                    qtile[0:64, :], start=True, stop=True,
                                    tile_position=(0, 0))
                                nc.tensor.matmul(
                                    sAB[:, TT:2 * TT], kpiece[64:128, ksl],
                                    qtile[64:128, :], start=True, stop=True,
                                    tile_position=(64, 0))
                                delta = j * KB - qi * TT
                                d0 = max(0, delta)
                                if 0 <= delta < TT:
                                    for half in (0, TT):
                                        bs = slice(half + delta,
                                                   half + delta + KB)
                                        nc.vector.tensor_add(
                                            sAB[:, bs], sAB[:, bs],
                                            mask_sb[:, :])
                                    for half in (0, TT):
                                        es = slice(half + d0, half + TT)
                                        nc.scalar.activation(
                                            ptAB[:, es], sAB[:, es], EXP,
                                            scale=SCALE)
                                else:
                                    nc.scalar.activation(
                                        ptAB[:, :], sAB[:, :], EXP,
                                        scale=SCALE)
                                tiles.append((j, d0, ptAB))
                            for j, d0, ptAB in tiles:
                                st, sp = (j == 0), (j == nkb - 1)
                                sub = slice(d0, TT)
                                vpc = vp[j // 4]
                                nc.tensor.matmul(
                                    ctxA[0:65, sub],
                                    vpc[:, j % 4, hA, 0:D + 1],
                                    ptAB[:, d0:TT], start=st, stop=sp)
                                nc.tensor.matmul(
                                    ctxB[0:65, sub],
                                    vpc[:, j % 4, hB, 0:D + 1],
                                    ptAB[:, TT + d0:2 * TT],
                                    start=st, stop=sp)
                        # evacuate unnormalized ctx + denominators
                        # (partition-shifted engine copies)
                        nc.vector.tensor_copy(ctx_sb[p][0:64, qsl],
                                              ctxA[0:64, :])
                        _cp(cfg["ctxB_evac"])(ctx_sb[p][64:128, qsl],
                                               ctxB[0:64, :])
                        _cp(cfg["denA_evac"])(den[qi][64 * p:64 * p + 1, :],
                                              ctxA[64:65, :])
                        _cp(cfg["denB_evac"])(den[qi]
                                              [64 * p + 32:64 * p + 33, :],
                                              ctxB[64:65, :])

                    def emit_normalize(qi):
                        qsl = slice(qi * TT, (qi + 1) * TT)
                        rec = npool.tile([128, TT], F32, tag="rec", bufs=2,
                                         name="rec")
                        nc.vector.reciprocal(rec[:, :], den[qi][:, :])
                        # bounce through DRAM to partition-broadcast
                        recd = dramp.tile([4, TT], F32, tag="recd",
                                          name="recd")
                        rec_r = rec.rearrange("(a b) f -> a b f", b=32)
                        nc.sync.dma_start(out=recd[0:4, :],
                                          in_=rec_r[0:4, 0, :])
                        for p in range(2):
                            rb = npool.tile([128, TT], F32, tag=f"rb{p}",
                                            bufs=2, name=f"rb{p}")
                            nc.sync.dma_start(
                                out=rb[0:64, :],
                                in_=recd[2 * p:2 * p + 1, :]
                                .partition_broadcast(64))
                            nc.sync.dma_start(
                                out=rb[64:128, :],
                                in_=recd[2 * p + 1:2 * p + 2, :]
                                .partition_broadcast(64))
                            eng("norm").tensor_mul(ctx_sb[p][:, qsl],
                                                   ctx_sb[p][:, qsl],
                                                   rb[:, :])

                    # interleave: QKV ttile tt, then attention qtile tt for
                    # both pairs (causality: qtile tt needs pieces <= tt only)
                    emit_x(0)
                    for tt in range(NTT):
                        if tt + 1 < NTT:
                            emit_x(tt + 1)
                        emit_qkv_ttile(tt)
                        emit_attention_qtile(0, tt)
                        emit_attention_qtile(1, tt)
                        emit_normalize(tt)

                # ---------------- Phase D: output projection ----------------
                with tc.tile_pool(name="yps", bufs=2, space="PSUM") as yps, \
                     tc.tile_pool(name="ysb", bufs=3) as ysb:
                    for tb in range(NKB):
                        tsl = slice(tb * 128, (tb + 1) * 128)
                        y_ps = yps.tile([128, 1024], F32, tag="y_ps")
                        for no in range(2):
                            osl = slice(no * 512, (no + 1) * 512)
                            for kb in range(2):
                                nc.tensor.matmul(
                                    y_ps[:, osl], ctx_sb[kb][:, tsl],
                                    wp_sb[:, kb, osl],
                                    start=(kb == 0), stop=(kb == 1))
                        y_sb = ysb.tile([128, 1024], F32, tag="y_sb")
                        nc.vector.tensor_copy(y_sb[:, 0:512], y_ps[:, 0:512])
                        nc.scalar.copy(y_sb[:, 512:1024], y_ps[:, 512:1024])
                        nc.sync.dma_start(out=y[tsl, :], in_=y_sb[:, :])

    return nc


# ---------------------------------------------------------------- host side
def _rope_tables():
    inv = 1.0 / (ROPE_BASE ** (np.arange(0, D, 2, dtype=np.float32) / D))  # [32]
    t = np.arange(T, dtype=np.float32)
    freqs = np.outer(t, inv)                      # [T, 32]
    cosF = np.empty((128, T), dtype=np.float32)
    sinF = np.empty((128, T), dtype=np.float32)
    for p_ in range(128):
        d = p_ % D
        cosF[p_] = np.cos(freqs[:, d % 32])
        s = np.sin(freqs[:, d % 32])
        sinF[p_] = -s if d < 32 else s
    return cosF, sinF


def _mask_band():
    jp = np.arange(128)[:, None]
    qb = np.arange(128)[None, :]
    return np.where(qb >= jp, 0.0, NEG).astype(np.float32)


def _np_dt(dt):
    import ml_dtypes
    return ml_dtypes.bfloat16 if dt == BF16 else np.float32


def prepare_in_maps(x, Wqkv, Wproj):
    x = np.asarray(x, dtype=np.float32)
    Wqkv = np.asarray(Wqkv, dtype=np.float32)
    Wproj = np.asarray(Wproj, dtype=np.float32)
    qkv_np, proj_np, att_np = _np_dt(QKV_DT), _np_dt(PROJ_DT), _np_dt(ATT_DT)
    cosF, sinF = _rope_tables()
    mb = _mask_band()
    xTs = [np.ascontiguousarray(x[b].T).astype(qkv_np) for b in range(B)]
    in_maps = []
    for core in range(N_CORES):
        b, g = divmod(core, 4)
        hs = [4 * g + i for i in range(HL)]
        q_rows = np.concatenate([Wqkv[h * D:(h + 1) * D] for h in hs])
        k_rows = np.concatenate([Wqkv[C + h * D:C + (h + 1) * D] for h in hs])
        v_rows = np.concatenate([Wqkv[2 * C + h * D:2 * C + (h + 1) * D] for h in hs])
        wqkT = np.ascontiguousarray(np.concatenate([q_rows, k_rows]).T).astype(qkv_np)
        wvT = np.ascontiguousarray(v_rows.T).astype(qkv_np)
        cols = np.concatenate([np.arange(h * D, (h + 1) * D) for h in hs])
        wpT = np.ascontiguousarray(Wproj[:, cols].T).astype(proj_np)
        in_maps.append(dict(xT=xTs[b], wqkT=wqkT, wvT=wvT, wpT=wpT,
                            cosF=cosF, sinF=sinF, maskband=mb,
                            vones=np.ones((1, NKB * HL), dtype=att_np)))
    return in_maps


_CACHE = {}


def _get_nc():
    if "nc" not in _CACHE:
        _CACHE["nc"] = build_bass()
    return _CACHE["nc"]


def _make_runner(nc, chain=1):
    """Jitted SPMD executor for an arbitrary prebuilt Bass module."""
    import jax
    from jax.experimental.shard_map import shard_map
    from jax.sharding import Mesh, PartitionSpec
    from concourse import bass2jax

    bass2jax.install_neuronx_cc_hook()

    import concourse.mybir as _mb
    partition_name = (nc.partition_id_tensor.name
                      if nc.partition_id_tensor else None)
    in_names, out_names, out_avals, zero_outs = [], [], [], []
    for alloc in nc.m.functions[0].allocations:
        if not isinstance(alloc, _mb.MemoryLocationSet):
            continue
        name = alloc.memorylocations[0].name
        if alloc.kind == "ExternalInput":
            if name != partition_name:
                in_names.append(name)
        elif alloc.kind == "ExternalOutput":
            out_names.append(name)
            shape = tuple(alloc.tensor_shape)
            dtype = _mb.dt.np(alloc.dtype)
            out_avals.append(jax.core.ShapedArray(shape, dtype))
            zero_outs.append(np.zeros(shape, dtype))
    n_params = len(in_names)
    all_in_names = in_names + out_names
    if partition_name is not None:
        all_in_names = all_in_names + [partition_name]

    def _body(*args):
        params = list(args[:n_params])
        bufs = list(args[n_params:])
        for _ in range(chain):
            operands = params + bufs
            if partition_name is not None:
                operands.append(bass2jax.partition_id_tensor())
            outs = bass2jax._bass_exec_p.bind(
                *operands,
                out_avals=tuple(out_avals),
                in_names=tuple(all_in_names),
                out_names=tuple(out_names),
                lowering_input_output_aliases=(),
                sim_require_finite=True,
                sim_require_nnan=True,
                nc=nc,
            )
            bufs = list(outs)
        return tuple(bufs)

    devices = jax.devices()[:N_CORES]
    mesh = Mesh(np.array(devices), ("core",))
    n_outs = len(out_names)
    in_specs = (PartitionSpec("core"),) * (n_params + n_outs)
    out_specs = (PartitionSpec("core"),) * n_outs
    fn = jax.jit(shard_map(_body, mesh=mesh, in_specs=in_specs,
                           out_specs=out_specs, check_rep=False),
                 keep_unused=True)
    return dict(fn=fn, in_names=in_names, out_names=out_names,
                out_avals=out_avals, zero_outs=zero_outs, n_params=n_params)


def _get_runner(chain=1):
    key = ("runner", chain)
    if key in _CACHE:
        return _CACHE[key]
    nc = _get_nc()
    if not _CACHE.get("legalized"):
        _legalize_sync_waits(nc, limit=1)
        _CACHE["legalized"] = True
    runner = _make_runner(nc, chain)
    _CACHE[key] = runner
    return runner


def _concat_args(runner, in_maps):
    concat_in = [np.concatenate([np.asarray(m[name]) for m in in_maps], axis=0)
                 for name in runner["in_names"]]
    concat_zeros = [np.zeros((N_CORES * z.shape[0], *z.shape[1:]), z.dtype)
                    for z in runner["zero_outs"]]
    return concat_in + concat_zeros


def _run(in_maps, chain=1, args=None):
    runner = _get_runner(chain)
    if args is None:
        args = _concat_args(runner, in_maps)
    out_arrs = runner["fn"](*args)
    results = []
    for c in range(N_CORES):
        results.append({
            name: np.asarray(out_arrs[i]).reshape(
                N_CORES, *runner["out_avals"][i].shape)[c]
            for i, name in enumerate(runner["out_names"])})
    return results


def kernel(x, Wqkv, Wproj):
    in_maps = prepare_in_maps(x, Wqkv, Wproj)
    results = _run(in_maps)
    out = np.zeros((B, T, C), dtype=np.float32)
    for core in range(N_CORES):
        b = core // 4
        out[b] += results[core]["y"]
    return out

